# revision 44
# baseline (speedup 1.0000x reference)
"""Trainium2 Bass kernel for GNN message passing:
    out[i] = sum_{e: dst[e]==i} x[src[e]]     (x: [N, 64] f32, edge_index: [2, E] int)

Strategy (node-sharded dst, 8 cores, aligned buckets):
  * Host assigns dst nodes to 8 cores x 99 tiles of 128 slots, balancing
    per-(tile, src-block) edge counts to <= 512 so every bucket is exactly
    4 chunks of 128 edges (1584 chunks/core, 1.3% over the 1563 floor);
    chunk boundaries never straddle tiles, so every chunk needs exactly
    ONE matmul (slots == chunks).
  * x is repacked as [N, 128] bf16 rows [hi | lo] (hi/lo split of f32):
    one 256 B gather per edge feeds one [128,128] bf16 matmul; hi+lo
    columns are merged after PSUM evacuation (~1e-5 relative accuracy).
  * dma_gather (GPSIMD SWDGE) is the bottleneck engine: calls are split
    into 32-chunk pieces round-robined over 4 SWDGE queues (the gather
    ucode runs on a pair of Q7 cpus per call; 4 queues = all 8 Q7 cores).
  * One-hot matrices are built on-chip by VectorE (is_equal of a repeated
    iota row against per-chunk dst columns, 8 slots per instruction) from
    a tiny resident ldst table — no one-hot DMA traffic.
  * TensorE: psum[tile] += onehot.T @ msgs, accumulated across the 4 src
    blocks of a supertile of 8 tiles (8 PSUM banks); ScalarE+VectorE merge
    hi+lo into a small per-supertile staging tile which is streamed to
    DRAM immediately (stores overlap compute); host un-permutes rows.
    No collectives.
"""

import numpy as np
import ml_dtypes

import concourse.bacc as bacc
import concourse.bass as bass
import concourse.mybir as mybir
import concourse.tile as tile
from concourse.bass_utils import run_bass_kernel_spmd

P = 128
F32 = mybir.dt.float32
BF16 = mybir.dt.bfloat16
I16 = mybir.dt.int16
I32 = mybir.dt.int32
BF = ml_dtypes.bfloat16

# Full-problem constants (hardcoded per harness contract).
N_NODES = 100000
DIM = 64
N_CORES = 8
SRC_BLOCK = 25000        # int16-safe source block
CHUNKS_PER_CALL = 32     # chunks per dma_gather call piece
TILES_PC = 99            # dst tiles per core (128 slots each, 12672 padded)
SUPERTILE = 8            # dst tiles with live PSUM banks
N_QUEUES = 4             # SWDGE queues (max 4)
SINGLE_PACKET = False
BUCKET_CAP = 512         # target per-(tile, block) edge count (4 chunks)


def _balance(deg, n_bins, cap, sweeps=10):
    """Assign nodes to n_bins bins (equal node counts) s.t. per-bin,
    per-block degree sums are (mostly) <= cap. Returns bin_of [N]."""
    n, nb = deg.shape
    total = deg.sum(1)
    order = np.argsort(-total, kind="stable")
    idx = np.arange(n)
    rows, cols = idx // n_bins, idx % n_bins
    cols = np.where(rows % 2 == 0, cols, n_bins - 1 - cols)
    bin_of = np.empty(n, np.int32)
    bin_of[order] = cols.astype(np.int32)

    loads = np.zeros((n_bins, nb), np.int64)
    np.add.at(loads, bin_of, deg)
    members = [list(np.where(bin_of == b)[0]) for b in range(n_bins)]

    for _ in range(sweeps):
        viol = np.argwhere(loads > cap)
        if len(viol) == 0:
            break
        for bb, blk in viol:
            tries = 0
            while loads[bb, blk] > cap and tries < 20:
                tries += 1
                mem = np.array(members[bb])
                nsel = mem[np.argmax(deg[mem, blk])]
                tgt = int(np.argmin(loads[:, blk]))
                if tgt == bb:
                    break
                tmem = np.array(members[tgt])
                msel = tmem[np.argmin(deg[tmem, blk])]
                if deg[msel, blk] >= deg[nsel, blk]:
                    break
                delta = deg[nsel] - deg[msel]
                if loads[tgt, blk] + delta[blk] > cap:
                    break
                loads[bb] -= delta
                loads[tgt] += delta
                members[bb].remove(nsel)
                members[bb].append(msel)
                members[tgt].remove(msel)
                members[tgt].append(nsel)
                bin_of[nsel] = tgt
                bin_of[msel] = bb
    return bin_of, loads, members


def _prep(edge_index, n_nodes, n_cores, block, w):
    tiles = TILES_PC
    nblocks = -(-n_nodes // block)
    n_bins = n_cores * tiles
    stile = SUPERTILE
    n_super = -(-tiles // stile)

    dst = np.asarray(edge_index[0]).astype(np.int64)
    src = np.asarray(edge_index[1]).astype(np.int64)
    blk_of = (src // block).astype(np.int64)

    deg = np.zeros((n_nodes, nblocks), np.int32)
    np.add.at(deg, (dst, blk_of), 1)

    bin_of, loads, members = _balance(deg, n_bins, BUCKET_CAP)

    # per-core slot ordering: heavy bins aligned across cores
    slot_of_bin = np.empty(n_bins, np.int64)
    bin_at = np.empty((n_cores, tiles), np.int64)
    for k in range(n_cores):
        bins_k = np.arange(k * tiles, (k + 1) * tiles)
        rank = np.argsort(-loads[bins_k].max(axis=1), kind="stable")
        slot_of_bin[bins_k[rank]] = np.arange(tiles)
        bin_at[k, np.arange(tiles)] = bins_k[rank]

    # chunks per (slot, block): shared across cores
    ld = loads.reshape(n_cores, tiles, nblocks)  # indexed by raw bin
    ld_slot = np.empty_like(ld)
    for k in range(n_cores):
        ld_slot[k] = ld[k][bin_at[k] - k * tiles]
    Q = np.maximum(-(-ld_slot.max(axis=0) // P), 1)  # [tiles, nblocks]

    # node -> (pos within its bin)
    node_pos = np.empty(n_nodes, np.int64)
    node_order = np.full((n_cores, tiles, P), -1, np.int64)
    for b in range(n_bins):
        mem = np.array(members[b], dtype=np.int64)
        node_pos[mem] = np.arange(len(mem))
        k = b // tiles
        s = slot_of_bin[b]
        node_order[k, s, : len(mem)] = mem

    # chunk layout in execution order: supertile -> block -> slot
    chunk_tile = []
    chunk_block = []
    bucket_c0 = np.zeros((tiles, nblocks), np.int64)
    calls = []  # (block, c0, csize, queue)
    for sti in range(n_super):
        ts = list(range(sti * stile, min((sti + 1) * stile, tiles)))
        for b in range(nblocks):
            g0 = len(chunk_tile)
            for s in ts:
                bucket_c0[s, b] = len(chunk_tile)
                chunk_tile += [s] * int(Q[s, b])
                chunk_block += [b] * int(Q[s, b])
            g1 = len(chunk_tile)
            c = g0
            while c < g1:
                csize = min(w, g1 - c)
                calls.append((b, c, csize))
                c += csize
    ch = len(chunk_tile)
    chunk_tile = np.array(chunk_tile)
    chunk_block = np.array(chunk_block)

    # first/last chunk per tile (for PSUM start/stop), scoped per supertile
    mm_first = np.zeros(ch, dtype=bool)
    mm_last = np.zeros(ch, dtype=bool)
    for sti in range(n_super):
        ts = range(sti * stile, min((sti + 1) * stile, tiles))
        for s in ts:
            cs = np.where(chunk_tile == s)[0]
            mm_first[cs[0]] = True
            mm_last[cs[-1]] = True

    # per-core streams
    idx_all = np.zeros((n_cores, P, ch * 8), np.int16)
    ldst_all = np.full((n_cores, P, ch), -1.0, BF)
    for k in range(n_cores):
        mask = (bin_of[dst] // tiles) == k
        ek = np.where(mask)[0]
        s_e = slot_of_bin[bin_of[dst[ek]]]
        b_e = blk_of[ek]
        key = s_e * nblocks + b_e
        order = np.argsort(key, kind="stable")
        ek = ek[order]
        s_e = s_e[order]
        b_e = b_e[order]
        # rank within bucket
        key_s = key[order]
        uniq, start_idx = np.unique(key_s, return_index=True)
        rank = np.arange(len(ek)) - np.repeat(start_idx, np.diff(
            np.append(start_idx, len(ek))))
        pos = bucket_c0[s_e, b_e] * P + rank
        idx_flat = np.zeros(ch * P, np.int16)
        idx_flat[pos] = (src[ek] - b_e * block).astype(np.int16)
        ohcol = np.full(ch * P, -1, np.int16)
        ohcol[pos] = node_pos[dst[ek]].astype(np.int16)
        # idx wrap: element j -> [j % 16, j // 16], replicated to 128 parts
        idx_all[k] = np.tile(
            idx_flat.reshape(ch * 8, 16).T, (8, 1)
        )
        # ldst[p, c] = one-hot column for edge slot p of chunk c (-1 = pad)
        ldst_all[k] = ohcol.reshape(ch, P).T.astype(BF)

    return dict(
        tiles=tiles,
        nblocks=nblocks,
        n_super=n_super,
        stile=stile,
        ch=ch,
        calls=calls,
        chunk_tile=chunk_tile,
        mm_first=mm_first,
        mm_last=mm_last,
        idx=idx_all,
        ldst=ldst_all,
        node_order=node_order,
        Q=Q,
    )


def _pack_x(x):
    """[N, D] f32 -> [N, 2D] bf16 rows: [hi | lo]."""
    x = np.asarray(x, np.float32)
    hi = x.astype(BF)
    lo = (x - hi.astype(np.float32)).astype(BF)
    return np.ascontiguousarray(np.concatenate([hi, lo], axis=1))


def _build(n_nodes, dim, block, w, sched):
    tiles = sched["tiles"]
    stile = sched["stile"]
    n_super = sched["n_super"]
    ch = sched["ch"]
    calls = sched["calls"]
    chunk_tile = sched["chunk_tile"]
    mm_first = sched["mm_first"]
    mm_last = sched["mm_last"]
    out_pad = tiles * P
    elem = 2 * dim  # packed bf16 row length

    nc = bacc.Bacc(
        "TRN2", target_bir_lowering=False, debug=False,
        num_swdge_queues=N_QUEUES,
    )
    x_t = nc.dram_tensor("xpack", [n_nodes, elem], BF16, kind="ExternalInput")
    idx_t = nc.dram_tensor("idx", [P, ch * 8], I16, kind="ExternalInput")
    ldst_t = nc.dram_tensor("ldst", [P, ch], BF16, kind="ExternalInput")
    out_t = nc.dram_tensor("out", [out_pad, dim], F32, kind="ExternalOutput")
    OHG = 8  # one-hot slots built per is_equal instruction

    with tile.TileContext(nc) as tc:
        with (
            tc.tile_pool(name="const", bufs=1) as const_pool,
            tc.tile_pool(name="meta", bufs=8) as meta_pool,
            tc.tile_pool(name="ohp", bufs=8) as oh_pool,
            tc.tile_pool(name="gather", bufs=8) as gather_pool,
            tc.tile_pool(name="stage", bufs=4) as stage_pool,
            tc.tile_pool(name="psum", bufs=8, space="PSUM") as psum_pool,
        ):
            iota_i = const_pool.tile([P, OHG * P], I32)
            nc.gpsimd.iota(
                iota_i[:], pattern=[[0, OHG], [1, P]], base=0,
                channel_multiplier=0,
            )
            iota_b = const_pool.tile([P, OHG * P], BF16)
            nc.vector.tensor_copy(iota_b[:], iota_i[:])
            # per-chunk one-hot columns, resident for the whole kernel
            ldst_sb = const_pool.tile([P, ch], BF16)
            nc.sync.dma_start(ldst_sb[:], ldst_t[:, :])

            call_idx = 0
            gather_q = 0
            psums = {}
            for sti in range(n_super):
                ts = list(range(sti * stile, min((sti + 1) * stile, tiles)))
                first_c = None
                while call_idx < len(calls):
                    b, c0, csize = calls[call_idx]
                    if chunk_tile[c0] not in ts:
                        break
                    call_idx += 1
                    idx_tile = meta_pool.tile([P, w * 8], I16, tag="idx")
                    nc.sync.dma_start(
                        idx_tile[:, : csize * 8],
                        idx_t[:, c0 * 8 : (c0 + csize) * 8],
                    )
                    oh_tile = oh_pool.tile([P, w, P], BF16, tag="oh")
                    for j0 in range(0, csize, OHG):
                        g = min(OHG, csize - j0)
                        lt = ldst_sb[:, c0 + j0 : c0 + j0 + g]
                        lt_b = bass.AP(lt.tensor, lt.offset, lt.ap + [[0, P]])
                        nc.vector.tensor_tensor(
                            out=oh_tile[:, j0 : j0 + g, :],
                            in0=iota_b[:, : g * P].rearrange(
                                "p (g q) -> p g q", q=P
                            ),
                            in1=lt_b,
                            op=mybir.AluOpType.is_equal,
                        )
                    msgs = gather_pool.tile([P, w, elem], BF16)
                    nc.gpsimd.dma_gather(
                        out_ap=msgs[:, :csize, :],
                        in_ap=x_t[b * block : min((b + 1) * block, n_nodes), :],
                        idxs_ap=idx_tile[:, : csize * 8],
                        num_idxs=csize * P,
                        num_idxs_reg=csize * P,
                        elem_size=elem,
                        single_packet=SINGLE_PACKET,
                        queue_num=gather_q,
                    )
                    gather_q = (gather_q + 1) % N_QUEUES
                    for j in range(csize):
                        c = c0 + j
                        t = int(chunk_tile[c])
                        if mm_first[c]:
                            psums[t] = psum_pool.tile(
                                [P, elem], F32, tag="ps", name=f"ps{t}"
                            )
                        nc.tensor.matmul(
                            psums[t][:, :],
                            lhsT=oh_tile[:, j, :],
                            rhs=msgs[:, j, :],
                            start=bool(mm_first[c]),
                            stop=bool(mm_last[c]),
                        )
                # evacuate: stage_s[:, j*dim:+dim] = psum_hi + psum_lo,
                # then stream this supertile's rows to DRAM immediately
                stage_s = stage_pool.tile([P, stile * dim], F32, tag="stg")
                for j, t in enumerate(ts):
                    if t not in psums:
                        continue
                    ps = psums.pop(t)
                    sl = stage_s[:, j * dim : (j + 1) * dim]
                    nc.scalar.copy(sl, ps[:, :dim])
                    nc.vector.tensor_tensor(
                        out=sl, in0=sl, in1=ps[:, dim:],
                        op=mybir.AluOpType.add,
                    )
                r0 = sti * stile * P
                nc.sync.dma_start(
                    out_t[r0 : r0 + len(ts) * P, :].rearrange(
                        "(t p) d -> p t d", p=P
                    ),
                    stage_s[:, : len(ts) * dim].rearrange(
                        "p (t d) -> p t d", d=dim
                    ),
                )

    nc.compile()
    return nc


def _run(x, edge_index, n_nodes, dim, n_cores, block, w, **run_kwargs):
    sched = _prep(edge_index, n_nodes, n_cores, block, w)
    xp = _pack_x(x)
    nc = _build(n_nodes, dim, block, w, sched)
    in_maps = [
        {"xpack": xp, "idx": sched["idx"][k], "ldst": sched["ldst"][k]}
        for k in range(n_cores)
    ]
    res = run_bass_kernel_spmd(
        nc, in_maps, core_ids=list(range(n_cores)), **run_kwargs
    )
    node_order = sched["node_order"]  # [cores, tiles, P]
    out = np.zeros((n_nodes, dim), np.float32)
    for k in range(n_cores):
        r = res.results[k]["out"]  # [tiles*P, dim]
        no = node_order[k].reshape(-1)
        m = no >= 0
        out[no[m]] = r[m]
    return out, res


def kernel(x, edge_index):
    out, _ = _run(
        x, edge_index, N_NODES, DIM, N_CORES, SRC_BLOCK, CHUNKS_PER_CALL
    )
    return out


# revision 48
# speedup vs baseline: 1.0166x; 1.0166x over previous
"""Trainium2 Bass kernel for GNN message passing:
    out[i] = sum_{e: dst[e]==i} x[src[e]]     (x: [N, 64] f32, edge_index: [2, E] int)

Strategy (node-sharded dst, 8 cores, aligned buckets):
  * Host assigns dst nodes to 8 cores x 99 tiles of 128 slots, balancing
    per-(tile, src-block) edge counts to <= 512 so every bucket is exactly
    4 chunks of 128 edges (1584 chunks/core, 1.3% over the 1563 floor);
    chunk boundaries never straddle tiles, so every chunk needs exactly
    ONE matmul (slots == chunks).
  * x is repacked as [N, 128] bf16 rows [hi | lo] (hi/lo split of f32):
    one 256 B gather per edge feeds one [128,128] bf16 matmul; hi+lo
    columns are merged after PSUM evacuation (~1e-5 relative accuracy).
  * dma_gather (GPSIMD SWDGE) is the bottleneck engine: calls are split
    into 32-chunk pieces round-robined over 4 SWDGE queues (the gather
    ucode runs on a pair of Q7 cpus per call; 4 queues = all 8 Q7 cores).
  * One-hot matrices are built on-chip by VectorE (is_equal of a repeated
    iota row against per-chunk dst columns, 8 slots per instruction) from
    a tiny resident ldst table — no one-hot DMA traffic.
  * TensorE: psum[tile] += onehot.T @ msgs, accumulated across the 4 src
    blocks of a supertile of 8 tiles (8 PSUM banks); ScalarE+VectorE merge
    hi+lo into a small per-supertile staging tile which is streamed to
    DRAM immediately (stores overlap compute); host un-permutes rows.
    No collectives.
"""

import numpy as np
import ml_dtypes

import concourse.bacc as bacc
import concourse.bass as bass
import concourse.mybir as mybir
import concourse.tile as tile
from concourse.bass_utils import run_bass_kernel_spmd

P = 128
F32 = mybir.dt.float32
BF16 = mybir.dt.bfloat16
I16 = mybir.dt.int16
I32 = mybir.dt.int32
BF = ml_dtypes.bfloat16

# Full-problem constants (hardcoded per harness contract).
N_NODES = 100000
DIM = 64
N_CORES = 8
SRC_BLOCK = 25000        # int16-safe source block
CHUNKS_PER_CALL = 32     # chunks per dma_gather call piece
TILES_PC = 99            # dst tiles per core (128 slots each, 12672 padded)
SUPERTILE = 8            # dst tiles with live PSUM banks
N_QUEUES = 4             # SWDGE queues (max 4)
SINGLE_PACKET = False
BUCKET_CAP = 512         # target per-(tile, block) edge count (4 chunks)
OHG = 8                  # one-hot slots built per is_equal instruction


def _balance(deg, n_bins, cap, sweeps=10):
    """Assign nodes to n_bins bins (equal node counts) s.t. per-bin,
    per-block degree sums are (mostly) <= cap. Returns bin_of [N]."""
    n, nb = deg.shape
    total = deg.sum(1)
    order = np.argsort(-total, kind="stable")
    idx = np.arange(n)
    rows, cols = idx // n_bins, idx % n_bins
    cols = np.where(rows % 2 == 0, cols, n_bins - 1 - cols)
    bin_of = np.empty(n, np.int32)
    bin_of[order] = cols.astype(np.int32)

    loads = np.zeros((n_bins, nb), np.int64)
    np.add.at(loads, bin_of, deg)
    members = [list(np.where(bin_of == b)[0]) for b in range(n_bins)]

    for _ in range(sweeps):
        viol = np.argwhere(loads > cap)
        if len(viol) == 0:
            break
        for bb, blk in viol:
            tries = 0
            while loads[bb, blk] > cap and tries < 20:
                tries += 1
                mem = np.array(members[bb])
                nsel = mem[np.argmax(deg[mem, blk])]
                tgt = int(np.argmin(loads[:, blk]))
                if tgt == bb:
                    break
                tmem = np.array(members[tgt])
                msel = tmem[np.argmin(deg[tmem, blk])]
                if deg[msel, blk] >= deg[nsel, blk]:
                    break
                delta = deg[nsel] - deg[msel]
                if loads[tgt, blk] + delta[blk] > cap:
                    break
                loads[bb] -= delta
                loads[tgt] += delta
                members[bb].remove(nsel)
                members[bb].append(msel)
                members[tgt].remove(msel)
                members[tgt].append(nsel)
                bin_of[nsel] = tgt
                bin_of[msel] = bb
    return bin_of, loads, members


def _prep(edge_index, n_nodes, n_cores, block, w):
    tiles = TILES_PC
    nblocks = -(-n_nodes // block)
    n_bins = n_cores * tiles
    stile = SUPERTILE
    n_super = -(-tiles // stile)

    dst = np.asarray(edge_index[0]).astype(np.int64)
    src = np.asarray(edge_index[1]).astype(np.int64)
    blk_of = (src // block).astype(np.int64)

    deg = np.zeros((n_nodes, nblocks), np.int32)
    np.add.at(deg, (dst, blk_of), 1)

    bin_of, loads, members = _balance(deg, n_bins, BUCKET_CAP)

    # per-core slot ordering: heavy bins aligned across cores
    slot_of_bin = np.empty(n_bins, np.int64)
    bin_at = np.empty((n_cores, tiles), np.int64)
    for k in range(n_cores):
        bins_k = np.arange(k * tiles, (k + 1) * tiles)
        rank = np.argsort(-loads[bins_k].max(axis=1), kind="stable")
        slot_of_bin[bins_k[rank]] = np.arange(tiles)
        bin_at[k, np.arange(tiles)] = bins_k[rank]

    # chunks per (slot, block): shared across cores
    ld = loads.reshape(n_cores, tiles, nblocks)  # indexed by raw bin
    ld_slot = np.empty_like(ld)
    for k in range(n_cores):
        ld_slot[k] = ld[k][bin_at[k] - k * tiles]
    Q = np.maximum(-(-ld_slot.max(axis=0) // P), 1)  # [tiles, nblocks]

    # node -> (pos within its bin)
    node_pos = np.empty(n_nodes, np.int64)
    node_order = np.full((n_cores, tiles, P), -1, np.int64)
    for b in range(n_bins):
        mem = np.array(members[b], dtype=np.int64)
        node_pos[mem] = np.arange(len(mem))
        k = b // tiles
        s = slot_of_bin[b]
        node_order[k, s, : len(mem)] = mem

    # chunk layout in execution order: supertile -> block -> slot
    chunk_tile = []
    chunk_block = []
    bucket_c0 = np.zeros((tiles, nblocks), np.int64)
    calls = []  # (block, c0, csize, queue)
    for sti in range(n_super):
        ts = list(range(sti * stile, min((sti + 1) * stile, tiles)))
        for b in range(nblocks):
            g0 = len(chunk_tile)
            for s in ts:
                bucket_c0[s, b] = len(chunk_tile)
                chunk_tile += [s] * int(Q[s, b])
                chunk_block += [b] * int(Q[s, b])
            g1 = len(chunk_tile)
            c = g0
            while c < g1:
                csize = min(w, g1 - c)
                calls.append((b, c, csize))
                c += csize
    ch = len(chunk_tile)
    chunk_tile = np.array(chunk_tile)
    chunk_block = np.array(chunk_block)

    # first/last chunk per tile (for PSUM start/stop), scoped per supertile
    mm_first = np.zeros(ch, dtype=bool)
    mm_last = np.zeros(ch, dtype=bool)
    for sti in range(n_super):
        ts = range(sti * stile, min((sti + 1) * stile, tiles))
        for s in ts:
            cs = np.where(chunk_tile == s)[0]
            mm_first[cs[0]] = True
            mm_last[cs[-1]] = True

    # per-core streams
    idx_all = np.zeros((n_cores, P, ch * 8), np.int16)
    ldst_all = np.full((n_cores, P, ch), -1.0, BF)
    for k in range(n_cores):
        mask = (bin_of[dst] // tiles) == k
        ek = np.where(mask)[0]
        s_e = slot_of_bin[bin_of[dst[ek]]]
        b_e = blk_of[ek]
        key = s_e * nblocks + b_e
        order = np.argsort(key, kind="stable")
        ek = ek[order]
        s_e = s_e[order]
        b_e = b_e[order]
        # rank within bucket
        key_s = key[order]
        uniq, start_idx = np.unique(key_s, return_index=True)
        rank = np.arange(len(ek)) - np.repeat(start_idx, np.diff(
            np.append(start_idx, len(ek))))
        pos = bucket_c0[s_e, b_e] * P + rank
        idx_flat = np.zeros(ch * P, np.int16)
        idx_flat[pos] = (src[ek] - b_e * block).astype(np.int16)
        ohcol = np.full(ch * P, -1, np.int16)
        ohcol[pos] = node_pos[dst[ek]].astype(np.int16)
        # idx wrap: element j -> [j % 16, j // 16], replicated to 128 parts
        idx_all[k] = np.tile(
            idx_flat.reshape(ch * 8, 16).T, (8, 1)
        )
        # ldst[p, c] = one-hot column for edge slot p of chunk c (-1 = pad)
        ldst_all[k] = ohcol.reshape(ch, P).T.astype(BF)

    return dict(
        tiles=tiles,
        nblocks=nblocks,
        n_super=n_super,
        stile=stile,
        ch=ch,
        calls=calls,
        chunk_tile=chunk_tile,
        mm_first=mm_first,
        mm_last=mm_last,
        idx=idx_all,
        ldst=ldst_all,
        node_order=node_order,
        Q=Q,
    )


def _pack_x(x):
    """[N, D] f32 -> [N, 2D] bf16 rows: [hi | lo]."""
    x = np.asarray(x, np.float32)
    hi = x.astype(BF)
    lo = (x - hi.astype(np.float32)).astype(BF)
    return np.ascontiguousarray(np.concatenate([hi, lo], axis=1))


def _build(n_nodes, dim, block, w, sched):
    tiles = sched["tiles"]
    stile = sched["stile"]
    n_super = sched["n_super"]
    ch = sched["ch"]
    calls = sched["calls"]
    chunk_tile = sched["chunk_tile"]
    mm_first = sched["mm_first"]
    mm_last = sched["mm_last"]
    out_pad = tiles * P
    elem = 2 * dim  # packed bf16 row length

    nc = bacc.Bacc(
        "TRN2", target_bir_lowering=False, debug=False,
        num_swdge_queues=N_QUEUES,
    )
    x_t = nc.dram_tensor("xpack", [n_nodes, elem], BF16, kind="ExternalInput")
    idx_t = nc.dram_tensor("idx", [P, ch * 8], I16, kind="ExternalInput")
    ldst_t = nc.dram_tensor("ldst", [P, ch], BF16, kind="ExternalInput")
    iota_t = nc.dram_tensor("iota", [P, OHG * P], BF16, kind="ExternalInput")
    out_t = nc.dram_tensor("out", [out_pad, dim], F32, kind="ExternalOutput")

    with tile.TileContext(nc) as tc:
        with (
            tc.tile_pool(name="const", bufs=1) as const_pool,
            tc.tile_pool(name="meta", bufs=8) as meta_pool,
            tc.tile_pool(name="ohp", bufs=8) as oh_pool,
            tc.tile_pool(name="gather", bufs=8) as gather_pool,
            tc.tile_pool(name="stage", bufs=4) as stage_pool,
            tc.tile_pool(name="psum", bufs=8, space="PSUM") as psum_pool,
        ):
            # host-shipped iota keeps GPSIMD preamble-free (gathers only)
            iota_b = const_pool.tile([P, OHG * P], BF16)
            nc.sync.dma_start(iota_b[:], iota_t[:, :])
            # per-chunk one-hot columns, resident for the whole kernel
            ldst_sb = const_pool.tile([P, ch], BF16)
            nc.sync.dma_start(ldst_sb[:], ldst_t[:, :])

            call_idx = 0
            gather_q = 0
            psums = {}
            for sti in range(n_super):
                ts = list(range(sti * stile, min((sti + 1) * stile, tiles)))
                first_c = None
                while call_idx < len(calls):
                    b, c0, csize = calls[call_idx]
                    if chunk_tile[c0] not in ts:
                        break
                    call_idx += 1
                    idx_tile = meta_pool.tile([P, w * 8], I16, tag="idx")
                    nc.sync.dma_start(
                        idx_tile[:, : csize * 8],
                        idx_t[:, c0 * 8 : (c0 + csize) * 8],
                    )
                    oh_tile = oh_pool.tile([P, w, P], BF16, tag="oh")
                    for j0 in range(0, csize, OHG):
                        g = min(OHG, csize - j0)
                        lt = ldst_sb[:, c0 + j0 : c0 + j0 + g]
                        lt_b = bass.AP(lt.tensor, lt.offset, lt.ap + [[0, P]])
                        nc.vector.tensor_tensor(
                            out=oh_tile[:, j0 : j0 + g, :],
                            in0=iota_b[:, : g * P].rearrange(
                                "p (g q) -> p g q", q=P
                            ),
                            in1=lt_b,
                            op=mybir.AluOpType.is_equal,
                        )
                    msgs = gather_pool.tile([P, w, elem], BF16)
                    nc.gpsimd.dma_gather(
                        out_ap=msgs[:, :csize, :],
                        in_ap=x_t[b * block : min((b + 1) * block, n_nodes), :],
                        idxs_ap=idx_tile[:, : csize * 8],
                        num_idxs=csize * P,
                        num_idxs_reg=csize * P,
                        elem_size=elem,
                        single_packet=SINGLE_PACKET,
                        queue_num=gather_q,
                    )
                    gather_q = (gather_q + 1) % N_QUEUES
                    for j in range(csize):
                        c = c0 + j
                        t = int(chunk_tile[c])
                        if mm_first[c]:
                            psums[t] = psum_pool.tile(
                                [P, elem], F32, tag="ps", name=f"ps{t}"
                            )
                        nc.tensor.matmul(
                            psums[t][:, :],
                            lhsT=oh_tile[:, j, :],
                            rhs=msgs[:, j, :],
                            start=bool(mm_first[c]),
                            stop=bool(mm_last[c]),
                        )
                # evacuate: stage_s[:, j*dim:+dim] = psum_hi + psum_lo,
                # then stream this supertile's rows to DRAM immediately
                stage_s = stage_pool.tile([P, stile * dim], F32, tag="stg")
                for j, t in enumerate(ts):
                    if t not in psums:
                        continue
                    ps = psums.pop(t)
                    sl = stage_s[:, j * dim : (j + 1) * dim]
                    nc.scalar.copy(sl, ps[:, :dim])
                    nc.vector.tensor_tensor(
                        out=sl, in0=sl, in1=ps[:, dim:],
                        op=mybir.AluOpType.add,
                    )
                r0 = sti * stile * P
                nc.sync.dma_start(
                    out_t[r0 : r0 + len(ts) * P, :].rearrange(
                        "(t p) d -> p t d", p=P
                    ),
                    stage_s[:, : len(ts) * dim].rearrange(
                        "p (t d) -> p t d", d=dim
                    ),
                )

    nc.compile()
    return nc


def _run(x, edge_index, n_nodes, dim, n_cores, block, w, **run_kwargs):
    sched = _prep(edge_index, n_nodes, n_cores, block, w)
    xp = _pack_x(x)
    nc = _build(n_nodes, dim, block, w, sched)
    iota_np = np.ascontiguousarray(
        np.broadcast_to(
            np.tile(np.arange(P, dtype=np.float32), OHG), (P, OHG * P)
        ).astype(BF)
    )
    in_maps = [
        {
            "xpack": xp,
            "idx": sched["idx"][k],
            "ldst": sched["ldst"][k],
            "iota": iota_np,
        }
        for k in range(n_cores)
    ]
    res = run_bass_kernel_spmd(
        nc, in_maps, core_ids=list(range(n_cores)), **run_kwargs
    )
    node_order = sched["node_order"]  # [cores, tiles, P]
    out = np.zeros((n_nodes, dim), np.float32)
    for k in range(n_cores):
        r = res.results[k]["out"]  # [tiles*P, dim]
        no = node_order[k].reshape(-1)
        m = no >= 0
        out[no[m]] = r[m]
    return out, res


def kernel(x, edge_index):
    out, _ = _run(
        x, edge_index, N_NODES, DIM, N_CORES, SRC_BLOCK, CHUNKS_PER_CALL
    )
    return out


# revision 50
# speedup vs baseline: 1.0313x; 1.0144x over previous
"""Trainium2 Bass kernel for GNN message passing:
    out[i] = sum_{e: dst[e]==i} x[src[e]]     (x: [N, 64] f32, edge_index: [2, E] int)

Strategy (node-sharded dst, 8 cores, aligned buckets):
  * Host assigns dst nodes to 8 cores x 99 tiles of 128 slots, balancing
    per-(tile, src-block) edge counts to <= 512 so every bucket is exactly
    4 chunks of 128 edges (1584 chunks/core, 1.3% over the 1563 floor);
    chunk boundaries never straddle tiles, so every chunk needs exactly
    ONE matmul (slots == chunks).
  * x is repacked as [N, 128] bf16 rows [hi | lo] (hi/lo split of f32):
    one 256 B gather per edge feeds one [128,128] bf16 matmul; hi+lo
    columns are merged after PSUM evacuation (~1e-5 relative accuracy).
  * dma_gather (GPSIMD SWDGE) is the bottleneck engine: calls are split
    into 32-chunk pieces round-robined over 4 SWDGE queues (the gather
    ucode runs on a pair of Q7 cpus per call; 4 queues = all 8 Q7 cores).
  * One-hot matrices are built on-chip by VectorE (is_equal of a repeated
    iota row against per-chunk dst columns, 8 slots per instruction) from
    a tiny resident ldst table — no one-hot DMA traffic.
  * TensorE: psum[tile] += onehot.T @ msgs, accumulated across the 4 src
    blocks of a supertile of 8 tiles (8 PSUM banks); ScalarE+VectorE merge
    hi+lo into a small per-supertile staging tile which is streamed to
    DRAM immediately (stores overlap compute); host un-permutes rows.
    No collectives.
"""

import numpy as np
import ml_dtypes

import concourse.bacc as bacc
import concourse.bass as bass
import concourse.mybir as mybir
import concourse.tile as tile
from concourse.bass_utils import run_bass_kernel_spmd

P = 128
F32 = mybir.dt.float32
BF16 = mybir.dt.bfloat16
I16 = mybir.dt.int16
I32 = mybir.dt.int32
BF = ml_dtypes.bfloat16

# Full-problem constants (hardcoded per harness contract).
N_NODES = 100000
DIM = 64
N_CORES = 8
SRC_BLOCK = 25000        # int16-safe source block
CHUNKS_PER_CALL = 32     # chunks per dma_gather call piece
TILES_PC = 99            # dst tiles per core (128 slots each, 12672 padded)
SUPERTILE = 8            # dst tiles with live PSUM banks
N_QUEUES = 4             # SWDGE queues (max 4)
SINGLE_PACKET = False
BUCKET_CAP = 512         # target per-(tile, block) edge count (4 chunks)
OHG = 8                  # one-hot slots built per is_equal instruction


def _balance(deg, n_bins, cap, sweeps=10):
    """Assign nodes to n_bins bins (equal node counts) s.t. per-bin,
    per-block degree sums are (mostly) <= cap. Returns bin_of [N]."""
    n, nb = deg.shape
    total = deg.sum(1)
    order = np.argsort(-total, kind="stable")
    idx = np.arange(n)
    rows, cols = idx // n_bins, idx % n_bins
    cols = np.where(rows % 2 == 0, cols, n_bins - 1 - cols)
    bin_of = np.empty(n, np.int32)
    bin_of[order] = cols.astype(np.int32)

    loads = np.zeros((n_bins, nb), np.int64)
    np.add.at(loads, bin_of, deg)
    members = [list(np.where(bin_of == b)[0]) for b in range(n_bins)]

    for _ in range(sweeps):
        viol = np.argwhere(loads > cap)
        if len(viol) == 0:
            break
        for bb, blk in viol:
            tries = 0
            while loads[bb, blk] > cap and tries < 20:
                tries += 1
                mem = np.array(members[bb])
                nsel = mem[np.argmax(deg[mem, blk])]
                tgt = int(np.argmin(loads[:, blk]))
                if tgt == bb:
                    break
                tmem = np.array(members[tgt])
                msel = tmem[np.argmin(deg[tmem, blk])]
                if deg[msel, blk] >= deg[nsel, blk]:
                    break
                delta = deg[nsel] - deg[msel]
                if loads[tgt, blk] + delta[blk] > cap:
                    break
                loads[bb] -= delta
                loads[tgt] += delta
                members[bb].remove(nsel)
                members[bb].append(msel)
                members[tgt].remove(msel)
                members[tgt].append(nsel)
                bin_of[nsel] = tgt
                bin_of[msel] = bb
    return bin_of, loads, members


def _prep(edge_index, n_nodes, n_cores, block, w):
    tiles = TILES_PC
    nblocks = -(-n_nodes // block)
    n_bins = n_cores * tiles
    stile = SUPERTILE
    n_super = -(-tiles // stile)

    dst = np.asarray(edge_index[0]).astype(np.int64)
    src = np.asarray(edge_index[1]).astype(np.int64)
    blk_of = (src // block).astype(np.int64)

    deg = np.zeros((n_nodes, nblocks), np.int32)
    np.add.at(deg, (dst, blk_of), 1)

    bin_of, loads, members = _balance(deg, n_bins, BUCKET_CAP)

    # per-core slot ordering: heavy bins aligned across cores
    slot_of_bin = np.empty(n_bins, np.int64)
    bin_at = np.empty((n_cores, tiles), np.int64)
    for k in range(n_cores):
        bins_k = np.arange(k * tiles, (k + 1) * tiles)
        rank = np.argsort(-loads[bins_k].max(axis=1), kind="stable")
        slot_of_bin[bins_k[rank]] = np.arange(tiles)
        bin_at[k, np.arange(tiles)] = bins_k[rank]

    # chunks per (slot, block): shared across cores
    ld = loads.reshape(n_cores, tiles, nblocks)  # indexed by raw bin
    ld_slot = np.empty_like(ld)
    for k in range(n_cores):
        ld_slot[k] = ld[k][bin_at[k] - k * tiles]
    Q = np.maximum(-(-ld_slot.max(axis=0) // P), 1)  # [tiles, nblocks]

    # node -> (pos within its bin)
    node_pos = np.empty(n_nodes, np.int64)
    node_order = np.full((n_cores, tiles, P), -1, np.int64)
    for b in range(n_bins):
        mem = np.array(members[b], dtype=np.int64)
        node_pos[mem] = np.arange(len(mem))
        k = b // tiles
        s = slot_of_bin[b]
        node_order[k, s, : len(mem)] = mem

    # chunk layout in execution order: supertile -> block -> slot
    chunk_tile = []
    chunk_block = []
    bucket_c0 = np.zeros((tiles, nblocks), np.int64)
    calls = []  # (block, c0, csize, queue)
    for sti in range(n_super):
        ts = list(range(sti * stile, min((sti + 1) * stile, tiles)))
        for b in range(nblocks):
            g0 = len(chunk_tile)
            for s in ts:
                bucket_c0[s, b] = len(chunk_tile)
                chunk_tile += [s] * int(Q[s, b])
                chunk_block += [b] * int(Q[s, b])
            g1 = len(chunk_tile)
            c = g0
            while c < g1:
                csize = min(w, g1 - c)
                calls.append((b, c, csize))
                c += csize
    ch = len(chunk_tile)
    chunk_tile = np.array(chunk_tile)
    chunk_block = np.array(chunk_block)

    # first/last chunk per tile (for PSUM start/stop), scoped per supertile
    mm_first = np.zeros(ch, dtype=bool)
    mm_last = np.zeros(ch, dtype=bool)
    for sti in range(n_super):
        ts = range(sti * stile, min((sti + 1) * stile, tiles))
        for s in ts:
            cs = np.where(chunk_tile == s)[0]
            mm_first[cs[0]] = True
            mm_last[cs[-1]] = True

    # per-core streams
    idx_all = np.zeros((n_cores, P, ch * 8), np.int16)
    ldst_all = np.full((n_cores, P, ch), -1.0, BF)
    for k in range(n_cores):
        mask = (bin_of[dst] // tiles) == k
        ek = np.where(mask)[0]
        s_e = slot_of_bin[bin_of[dst[ek]]]
        b_e = blk_of[ek]
        key = s_e * nblocks + b_e
        order = np.argsort(key, kind="stable")
        ek = ek[order]
        s_e = s_e[order]
        b_e = b_e[order]
        # rank within bucket
        key_s = key[order]
        uniq, start_idx = np.unique(key_s, return_index=True)
        rank = np.arange(len(ek)) - np.repeat(start_idx, np.diff(
            np.append(start_idx, len(ek))))
        pos = bucket_c0[s_e, b_e] * P + rank
        idx_flat = np.zeros(ch * P, np.int16)
        idx_flat[pos] = (src[ek] - b_e * block).astype(np.int16)
        ohcol = np.full(ch * P, -1, np.int16)
        ohcol[pos] = node_pos[dst[ek]].astype(np.int16)
        # idx wrap: element j -> [j % 16, j // 16], replicated to 128 parts
        idx_all[k] = np.tile(
            idx_flat.reshape(ch * 8, 16).T, (8, 1)
        )
        # ldst[p, c] = one-hot column for edge slot p of chunk c (-1 = pad)
        ldst_all[k] = ohcol.reshape(ch, P).T.astype(BF)

    return dict(
        tiles=tiles,
        nblocks=nblocks,
        n_super=n_super,
        stile=stile,
        ch=ch,
        calls=calls,
        chunk_tile=chunk_tile,
        mm_first=mm_first,
        mm_last=mm_last,
        idx=idx_all,
        ldst=ldst_all,
        node_order=node_order,
        Q=Q,
    )


def _pack_x(x):
    """[N, D] f32 -> [N, 2D] bf16 rows: [hi | lo]."""
    x = np.asarray(x, np.float32)
    hi = x.astype(BF)
    lo = (x - hi.astype(np.float32)).astype(BF)
    return np.ascontiguousarray(np.concatenate([hi, lo], axis=1))


def _build(n_nodes, dim, block, w, sched):
    tiles = sched["tiles"]
    stile = sched["stile"]
    n_super = sched["n_super"]
    ch = sched["ch"]
    calls = sched["calls"]
    chunk_tile = sched["chunk_tile"]
    mm_first = sched["mm_first"]
    mm_last = sched["mm_last"]
    out_pad = tiles * P
    elem = 2 * dim  # packed bf16 row length

    nc = bacc.Bacc(
        "TRN2", target_bir_lowering=False, debug=False,
        num_swdge_queues=N_QUEUES,
    )
    x_t = nc.dram_tensor("xpack", [n_nodes, elem], BF16, kind="ExternalInput")
    idx_t = nc.dram_tensor("idx", [P, ch * 8], I16, kind="ExternalInput")
    ldst_t = nc.dram_tensor("ldst", [P, ch], BF16, kind="ExternalInput")
    iota_t = nc.dram_tensor("iota", [P, OHG * P], BF16, kind="ExternalInput")
    out_t = nc.dram_tensor("out", [out_pad, dim], F32, kind="ExternalOutput")

    with tile.TileContext(nc) as tc:
        with (
            tc.tile_pool(name="const", bufs=1) as const_pool,
            tc.tile_pool(name="meta", bufs=8) as meta_pool,
            tc.tile_pool(name="ohp", bufs=8) as oh_pool,
            tc.tile_pool(name="gather", bufs=8) as gather_pool,
            tc.tile_pool(name="stage", bufs=4) as stage_pool,
            tc.tile_pool(name="psum", bufs=8, space="PSUM") as psum_pool,
        ):
            # host-shipped iota keeps GPSIMD preamble-free (gathers only)
            iota_b = const_pool.tile([P, OHG * P], BF16)
            nc.sync.dma_start(iota_b[:], iota_t[:, :])
            # per-chunk one-hot columns, resident for the whole kernel
            ldst_sb = const_pool.tile([P, ch], BF16)
            nc.sync.dma_start(ldst_sb[:], ldst_t[:, :])

            call_idx = 0
            gather_q = 0
            psums = {}
            oh_tiles = {}

            # one-hot builds are pipelined one supertile (4 calls) ahead of
            # their matmuls so evac adds waiting at the DVE queue head never
            # block the builds the next supertile's matmuls need
            def build_oh(i):
                b_, c0_, csize_ = calls[i]
                t_ = oh_pool.tile([P, w, P], BF16, tag="oh", name=f"oh{i}")
                for j0 in range(0, csize_, OHG):
                    g = min(OHG, csize_ - j0)
                    lt = ldst_sb[:, c0_ + j0 : c0_ + j0 + g]
                    lt_b = bass.AP(lt.tensor, lt.offset, lt.ap + [[0, P]])
                    nc.vector.tensor_tensor(
                        out=t_[:, j0 : j0 + g, :],
                        in0=iota_b[:, : g * P].rearrange(
                            "p (g q) -> p g q", q=P
                        ),
                        in1=lt_b,
                        op=mybir.AluOpType.is_equal,
                    )
                oh_tiles[i] = t_

            for i in range(min(4, len(calls))):
                build_oh(i)

            for sti in range(n_super):
                ts = list(range(sti * stile, min((sti + 1) * stile, tiles)))
                while call_idx < len(calls):
                    b, c0, csize = calls[call_idx]
                    if chunk_tile[c0] not in ts:
                        break
                    call_idx += 1
                    idx_tile = meta_pool.tile([P, w * 8], I16, tag="idx")
                    nc.sync.dma_start(
                        idx_tile[:, : csize * 8],
                        idx_t[:, c0 * 8 : (c0 + csize) * 8],
                    )
                    oh_tile = oh_tiles.pop(call_idx - 1)
                    msgs = gather_pool.tile([P, w, elem], BF16)
                    nc.gpsimd.dma_gather(
                        out_ap=msgs[:, :csize, :],
                        in_ap=x_t[b * block : min((b + 1) * block, n_nodes), :],
                        idxs_ap=idx_tile[:, : csize * 8],
                        num_idxs=csize * P,
                        num_idxs_reg=csize * P,
                        elem_size=elem,
                        single_packet=SINGLE_PACKET,
                        queue_num=gather_q,
                    )
                    gather_q = (gather_q + 1) % N_QUEUES
                    if call_idx - 1 + 4 < len(calls):
                        build_oh(call_idx - 1 + 4)
                    for j in range(csize):
                        c = c0 + j
                        t = int(chunk_tile[c])
                        if mm_first[c]:
                            psums[t] = psum_pool.tile(
                                [P, elem], F32, tag="ps", name=f"ps{t}"
                            )
                        nc.tensor.matmul(
                            psums[t][:, :],
                            lhsT=oh_tile[:, j, :],
                            rhs=msgs[:, j, :],
                            start=bool(mm_first[c]),
                            stop=bool(mm_last[c]),
                        )
                # evacuate: stage_s[:, j*dim:+dim] = psum_hi + psum_lo,
                # then stream this supertile's rows to DRAM immediately
                stage_s = stage_pool.tile([P, stile * dim], F32, tag="stg")
                for j, t in enumerate(ts):
                    if t not in psums:
                        continue
                    ps = psums.pop(t)
                    sl = stage_s[:, j * dim : (j + 1) * dim]
                    nc.scalar.copy(sl, ps[:, :dim])
                    nc.vector.tensor_tensor(
                        out=sl, in0=sl, in1=ps[:, dim:],
                        op=mybir.AluOpType.add,
                    )
                r0 = sti * stile * P
                nc.sync.dma_start(
                    out_t[r0 : r0 + len(ts) * P, :].rearrange(
                        "(t p) d -> p t d", p=P
                    ),
                    stage_s[:, : len(ts) * dim].rearrange(
                        "p (t d) -> p t d", d=dim
                    ),
                )

    nc.compile()
    return nc


def _run(x, edge_index, n_nodes, dim, n_cores, block, w, **run_kwargs):
    sched = _prep(edge_index, n_nodes, n_cores, block, w)
    xp = _pack_x(x)
    nc = _build(n_nodes, dim, block, w, sched)
    iota_np = np.ascontiguousarray(
        np.broadcast_to(
            np.tile(np.arange(P, dtype=np.float32), OHG), (P, OHG * P)
        ).astype(BF)
    )
    in_maps = [
        {
            "xpack": xp,
            "idx": sched["idx"][k],
            "ldst": sched["ldst"][k],
            "iota": iota_np,
        }
        for k in range(n_cores)
    ]
    res = run_bass_kernel_spmd(
        nc, in_maps, core_ids=list(range(n_cores)), **run_kwargs
    )
    node_order = sched["node_order"]  # [cores, tiles, P]
    out = np.zeros((n_nodes, dim), np.float32)
    for k in range(n_cores):
        r = res.results[k]["out"]  # [tiles*P, dim]
        no = node_order[k].reshape(-1)
        m = no >= 0
        out[no[m]] = r[m]
    return out, res


def kernel(x, edge_index):
    out, _ = _run(
        x, edge_index, N_NODES, DIM, N_CORES, SRC_BLOCK, CHUNKS_PER_CALL
    )
    return out
